# revision 1
# baseline (speedup 1.0000x reference)
"""Trainium2 Bass kernel for nn_CPCircuitLayer.

Math: with all_indices the full cartesian grid (s = n // H, h = n % H),
    out[b, s, h] = sum_r seq_emb[b,s,r] * hid_emb[b,h,r] * cp[r]
                 = (seq_emb[b] @ diag(cp) @ hid_emb[b].T)[s, h]
where seq_emb[b] = X_b @ seq_W.T  (X_b = hidden_states[b], contract H)
      hid_emb[b] = X_b.T @ hid_W.T                        (contract S)

Sharding: 8 cores = (batch b, seq half) pairs. Each core receives X_b
fully (the hid factor contracts over all of S) with rows rotated so its
own seq half comes first, plus a host-transposed copy of that half
(xt = X_b[half].T), and computes
    hid_embT = (hid_W*cp) @ X_b          [R, H]
    seq_embT = seq_W @ X_b[half].T       [R, S/2]
    out_half = seq_embT.T @ hid_embT     [S/2, H]
writing its [512, 1024] slice of the output.

The device program is raw Bass (no Tile framework) with manual
semaphores: x tiles stream on the Sync HWDGE queue and xt tiles on the
Activation HWDGE queue in parallel, the PE consumes tiles as they
arrive (hid + seq interleaved), PSUM->SBUF copies alternate between the
Vector and Scalar engines, and the 8 output chunks DMA out through a
5-deep PSUM bank rotation. Matmuls run in FP32R (fp32 rounded to 12
mantissa bits, streamed at full PE rate); inputs are pre-rounded to the
FP32R bit format on the host so the device does no conversion work. A
few dummy matmuls at kernel start warm the PE HAM clock gate.
"""

import numpy as np

B, S, H, R = 4, 1024, 1024, 32
N_CORES = 8
SH = S // 2   # seq rows per core
KT = S // 128  # k-tiles over the contraction dims
MT = SH // 128  # row tiles in this core's seq half

_compiled = {}


def _np_fallback(hidden_states, all_indices, seq_W, hid_W, cp_weight):
    seq_emb = np.einsum("bsh,rh->bsr", hidden_states, seq_W)
    hid_emb = np.einsum("bsh,rs->bhr", hidden_states, hid_W)
    s_idx = all_indices[:, 0].astype(np.int64)
    h_idx = all_indices[:, 1].astype(np.int64)
    g_seq = seq_emb[:, s_idx, :]
    g_hid = hid_emb[:, h_idx, :]
    out = np.einsum("bnr,bnr,r->bn", g_seq, g_hid, cp_weight[0])
    return out.reshape(B, S, H).astype(np.float32)


def _round_f32r(a):
    """Round fp32 to the FP32R format (RNE at 12 mantissa bits), bit-exact
    with the device's own fp32->fp32r conversion."""
    b = np.ascontiguousarray(a, dtype=np.float32).view(np.uint32)
    r = (b + np.uint32(0x7FF) + ((b >> np.uint32(12)) & np.uint32(1))) \
        & np.uint32(0xFFFFF000)
    return r.view(np.float32)


def _wtile(w):
    """[K, R] -> [128, KT*R] tile layout, partition-contiguous."""
    return np.ascontiguousarray(
        w.reshape(KT, 128, R).transpose(1, 0, 2).reshape(128, KT * R))


def build_raw_program():
    import contextlib

    import concourse.bass as bass
    import concourse.mybir as mybir

    f32 = mybir.dt.float32
    f32r = mybir.dt.float32r

    nc = bass.Bass("TRN2", target_bir_lowering=False, debug=False,
                   num_devices=N_CORES, enable_partition_id=False)

    x_d = nc.dram_tensor("x", [S, H], f32r, kind="ExternalInput")
    xt_d = nc.dram_tensor("xt", [H, SH], f32r, kind="ExternalInput")
    w_d = nc.dram_tensor("w", [128, 2 * KT * R], f32r, kind="ExternalInput")
    out_d = nc.dram_tensor("out", [SH, H], f32, kind="ExternalOutput")

    with contextlib.ExitStack() as _xs:
        E = _xs.enter_context
        w_t = E(nc.sbuf_tensor([128, 2 * KT * R], f32r))  # [p, sw | hw]
        x_t = E(nc.sbuf_tensor([128, KT, H], f32r))
        xt_t = E(nc.sbuf_tensor([128, KT, SH], f32r))
        hid_sb = E(nc.sbuf_tensor([R, H], f32r))
        seq_sb = E(nc.sbuf_tensor([R, SH], f32r))
        o_sb = E(nc.sbuf_tensor([128, MT, H], f32))
        hid_ps = E(nc.psum_tensor([R, H], f32))        # 2 banks
        seq_ps = E(nc.psum_tensor([R, SH], f32))       # 1 bank
        o_ps = [E(nc.psum_tensor(f"o_ps{i}", [128, 512], f32))
                for i in range(5)]                     # 5 banks
        dma_sem = E(nc.semaphore("dma_sem"))
        w_sem = E(nc.semaphore("w_sem"))
        pe_sem = E(nc.semaphore("pe_sem"))
        dve_sem = E(nc.semaphore("dve_sem"))
        act_sem = E(nc.semaphore("act_sem"))
        x_sem = [E(nc.semaphore(f"x_sem{j}")) for j in range(KT)]
        xt_sem = [E(nc.semaphore(f"xt_sem{j}")) for j in range(KT)]
        block = E(nc.Block(no_gpsimd_drain=True))

        sw = lambda k: w_t.ap()[:, k * R:(k + 1) * R]
        hw = lambda k: w_t.ap()[:, KT * R + k * R:KT * R + (k + 1) * R]

        # dve ops: 1 = hid_c0, 2 = seq_c, 3.. = even out copies
        # act ops: 1 = hid_c1, 2.. = odd out copies

        @block.sync
        def _(sync):
            sync.dma_start(out=w_t.ap(), in_=w_d[:]).then_inc(w_sem, 16)
            for k in range(KT):
                sync.dma_start(
                    out=x_t.ap()[:, k, :],
                    in_=x_d[k * 128:(k + 1) * 128, :],
                ).then_inc(x_sem[k], 16)
            # even out chunks: dispatch once the DVE copy lands in o_sb
            for j in range(0, 2 * MT, 2):
                m, n = divmod(j, 2)
                sync.wait_ge(dve_sem, 3 + j // 2)
                sync.dma_start(
                    out=out_d[m * 128:(m + 1) * 128, n * 512:(n + 1) * 512],
                    in_=o_sb.ap()[:, m, n * 512:(n + 1) * 512],
                ).then_inc(dma_sem, 16)
            sync.wait_ge(dma_sem, 16 * 2 * MT)

        @block.tensor
        def _(tensor):
            # pe counts: per k: hid n0 = 3k+1, n1 = 3k+2, seq = 3k+3
            # hid_n0 done @22, n1 @23, seq @24; final: 25..32
            tensor.wait_ge(w_sem, 16)
            for _ in range(6):
                nc.tensor.matmul(o_ps[0].ap()[0:R, :], w_t.ap()[:, 0:R],
                                 w_t.ap()[:, 0:512], start=True, stop=True)

            for k in range(KT):
                tensor.wait_ge(x_sem[k], 16)
                for n in range(2):
                    nc.tensor.matmul(
                        hid_ps.ap()[:, n * 512:(n + 1) * 512],
                        hw(k), x_t.ap()[:, k, n * 512:(n + 1) * 512],
                        start=(k == 0), stop=(k == KT - 1),
                    ).then_inc(pe_sem, 1)
                tensor.wait_ge(xt_sem[k], 16)
                nc.tensor.matmul(
                    seq_ps.ap(), sw(k), xt_t.ap()[:, k, :],
                    start=(k == 0), stop=(k == KT - 1),
                ).then_inc(pe_sem, 1)
                if k >= 4 and k < KT - 1:
                    # keep the PE HAM activity window busy into the final burst
                    nc.tensor.matmul(o_ps[4].ap()[0:R, :], w_t.ap()[:, 0:R],
                                     w_t.ap()[:, 0:512], start=True, stop=True)

            tensor.wait_ge(dve_sem, 2)   # hid_c0 + seq_c
            for j in range(2 * MT):
                m, n = divmod(j, 2)
                if j == 1:
                    tensor.wait_ge(act_sem, 1)   # hid_c1 (odd j only)
                if j >= 5:
                    # WAR on recycled PSUM bank (5-deep rotation)
                    prev = j - 5
                    if prev % 2 == 0:
                        tensor.wait_ge(dve_sem, 3 + prev // 2)
                    else:
                        tensor.wait_ge(act_sem, 2 + (prev - 1) // 2)
                nc.tensor.matmul(
                    o_ps[j % 5].ap(),
                    seq_sb.ap()[:, m * 128:(m + 1) * 128],
                    hid_sb.ap()[:, n * 512:(n + 1) * 512],
                    start=True, stop=True,
                ).then_inc(pe_sem, 1)

        @block.vector
        def _(vector):
            vector.wait_ge(pe_sem, 22)
            nc.vector.tensor_copy(
                hid_sb.ap()[:, 0:512],
                hid_ps.ap()[:, 0:512].bitcast(f32)).then_inc(dve_sem, 1)
            vector.wait_ge(pe_sem, 24)
            nc.vector.tensor_copy(
                seq_sb.ap(), seq_ps.ap().bitcast(f32)).then_inc(dve_sem, 1)
            for j in range(0, 2 * MT, 2):   # even out copies
                m, n = divmod(j, 2)
                vector.wait_ge(pe_sem, 24 + j + 1)
                nc.vector.tensor_copy(
                    o_sb.ap()[:, m, n * 512:(n + 1) * 512],
                    o_ps[j % 5].ap(),
                ).then_inc(dve_sem, 1)

        @block.scalar
        def _(scalar):
            # xt loads dispatch from the second HWDGE queue (Activation)
            for k in range(KT):
                scalar.dma_start(
                    out=xt_t.ap()[:, k, :],
                    in_=xt_d[k * 128:(k + 1) * 128, :],
                ).then_inc(xt_sem[k], 16)
            # dummy copy to pull the lazy ACT table load off the critical path
            scalar.wait_ge(w_sem, 16)
            nc.scalar.copy(o_sb.ap()[:, 0, 0:R], w_t.ap()[:, 0:R])
            scalar.wait_ge(pe_sem, 23)
            nc.scalar.copy(
                hid_sb.ap()[:, 512:1024],
                hid_ps.ap()[:, 512:1024].bitcast(f32)).then_inc(act_sem, 1)
            for j in range(1, 2 * MT, 2):   # odd out copies + own-queue DMA
                m, n = divmod(j, 2)
                scalar.wait_ge(pe_sem, 24 + j + 1)
                nc.scalar.copy(
                    o_sb.ap()[:, m, n * 512:(n + 1) * 512],
                    o_ps[j % 5].ap(),
                ).then_inc(act_sem, 1)
                scalar.dma_start(
                    out=out_d[m * 128:(m + 1) * 128, n * 512:(n + 1) * 512],
                    in_=o_sb.ap()[:, m, n * 512:(n + 1) * 512],
                ).then_inc(dma_sem, 16)

    return nc


def _get_program():
    if "nc" not in _compiled:
        _compiled["nc"] = build_raw_program()
    return _compiled["nc"]


def _make_in_maps(hidden_states, seq_W, hid_W, cp_weight):
    swT = _wtile(np.ascontiguousarray(seq_W.T))                    # [128, 256]
    hwT_rows = np.ascontiguousarray((hid_W * cp_weight[0][:, None]).T)  # [S, R]
    # per-half row rotation: own seq half first (hid contraction over S is
    # order-invariant as long as x rows and hw rows permute together)
    w_rot = [
        _round_f32r(np.concatenate([swT, _wtile(np.concatenate(
            [hwT_rows[half * SH:], hwT_rows[:half * SH]], axis=0))], axis=1))
        for half in range(2)
    ]
    in_maps = []
    for c in range(N_CORES):
        b, half = divmod(c, 2)
        xb = _round_f32r(hidden_states[b])
        if half:
            xb = np.ascontiguousarray(
                np.concatenate([xb[SH:], xb[:SH]], axis=0))
        in_maps.append({
            "x": xb,
            "xt": np.ascontiguousarray(xb[:SH, :].T),
            "w": w_rot[half],
        })
    return in_maps


def kernel(hidden_states, all_indices, seq_W, hid_W, cp_weight):
    hidden_states = np.asarray(hidden_states, dtype=np.float32)
    seq_W = np.asarray(seq_W, dtype=np.float32)
    hid_W = np.asarray(hid_W, dtype=np.float32)
    cp_weight = np.asarray(cp_weight, dtype=np.float32)
    idx = np.asarray(all_indices)

    # The reference's all_indices is always the full cartesian grid; verify
    # cheaply and fall back to a host path if ever not.
    n = np.arange(S * H, dtype=idx.dtype)
    if idx.shape != (S * H, 2) or not (
        np.array_equal(idx[:, 0], n // H) and np.array_equal(idx[:, 1], n % H)
    ):
        return _np_fallback(hidden_states, idx, seq_W, hid_W, cp_weight)

    from concourse.bass_utils import run_bass_kernel_spmd

    nc = _get_program()
    in_maps = _make_in_maps(hidden_states, seq_W, hid_W, cp_weight)
    res = run_bass_kernel_spmd(nc, in_maps, list(range(N_CORES)))

    out = np.empty((B, S, H), dtype=np.float32)
    for c in range(N_CORES):
        b, half = divmod(c, 2)
        out[b, half * SH:(half + 1) * SH, :] = res.results[c]["out"]
    return out



# revision 3
# speedup vs baseline: 1.3137x; 1.3137x over previous
"""Trainium2 Bass kernel for nn_CPCircuitLayer.

Math: with all_indices the full cartesian grid (s = n // H, h = n % H),
    out[b, s, h] = sum_r seq_emb[b,s,r] * hid_emb[b,h,r] * cp[r]
                 = (seq_emb[b] @ diag(cp) @ hid_emb[b].T)[s, h]
where seq_emb[b] = X_b @ seq_W.T  (X_b = hidden_states[b], contract H)
      hid_emb[b] = X_b.T @ hid_W.T                        (contract S)

Sharding: 8 cores = (batch b, seq half) pairs. Each core receives X_b
fully (the hid factor contracts over all of S) with rows rotated so its
own seq half comes first, plus a host-transposed copy of that half
(xt = X_b[half].T), and computes
    hid_embT = (hid_W*cp) @ X_b          [R, H]
    seq_embT = seq_W @ X_b[half].T       [R, S/2]
    out_half = seq_embT.T @ hid_embT     [S/2, H]
writing its [512, 1024] slice of the output.

v2: everything on device is float16 (inputs, matmuls, output), which
halves DMA traffic vs fp32r and streams the PE at 1 cycle/row instead
of ~3. All DRAM tensors are host-pre-tiled to [128, ...] partition
layout so every DMA moves >=2KB contiguous per partition row, and the
transfer count drops from 25 to 10 (w, 4x x-chunks, 2x xt-chunks,
3... 4 out chunks) balanced across the two HWDGE queues (Sync +
Activation). The fp16 output is upcast to f32 on the host. Total
rounding error ~1e-3 relative, far under the 2e-2 gate.
"""

import numpy as np

B, S, H, R = 4, 1024, 1024, 32
N_CORES = 8
SH = S // 2    # seq rows per core
KT = S // 128  # k-tiles over the contraction dims
MT = SH // 128  # row tiles in this core's seq half

_compiled = {}


def _np_fallback(hidden_states, all_indices, seq_W, hid_W, cp_weight):
    seq_emb = np.einsum("bsh,rh->bsr", hidden_states, seq_W)
    hid_emb = np.einsum("bsh,rs->bhr", hidden_states, hid_W)
    s_idx = all_indices[:, 0].astype(np.int64)
    h_idx = all_indices[:, 1].astype(np.int64)
    g_seq = seq_emb[:, s_idx, :]
    g_hid = hid_emb[:, h_idx, :]
    out = np.einsum("bnr,bnr,r->bn", g_seq, g_hid, cp_weight[0])
    return out.reshape(B, S, H).astype(np.float32)


def _tile128(a):
    """[K*128, N] -> [128, K*N] with k-tiles adjacent in the free dim."""
    k = a.shape[0] // 128
    return np.ascontiguousarray(
        a.reshape(k, 128, a.shape[1]).transpose(1, 0, 2).reshape(128, -1))


def _wtile(w):
    """[K, R] -> [128, KT*R] tile layout, partition-contiguous."""
    return np.ascontiguousarray(
        w.reshape(KT, 128, R).transpose(1, 0, 2).reshape(128, KT * R))


def build_raw_program():
    import contextlib

    import concourse.bass as bass
    import concourse.mybir as mybir

    f32 = mybir.dt.float32
    f16 = mybir.dt.float16

    nc = bass.Bass("TRN2", target_bir_lowering=False, debug=False,
                   num_devices=N_CORES, enable_partition_id=False)

    x_d = nc.dram_tensor("x", [128, KT * H], f16, kind="ExternalInput")
    xt_d = nc.dram_tensor("xt", [128, KT * SH], f16, kind="ExternalInput")
    w_d = nc.dram_tensor("w", [128, 2 * KT * R], f16, kind="ExternalInput")
    out_d = nc.dram_tensor("out", [128, MT * H], f16, kind="ExternalOutput")

    with contextlib.ExitStack() as _xs:
        E = _xs.enter_context
        w_t = E(nc.sbuf_tensor([128, 2 * KT * R], f16))  # [p, sw | hw]
        x_t = E(nc.sbuf_tensor([128, KT, H], f16))
        xt_t = E(nc.sbuf_tensor([128, KT, SH], f16))
        hid_sb = E(nc.sbuf_tensor([R, H], f16))
        seq_sb = E(nc.sbuf_tensor([R, SH], f16))
        o_sb = E(nc.sbuf_tensor([128, MT, H], f16))
        hid_ps = E(nc.psum_tensor([R, H], f32))        # 2 banks
        seq_ps = E(nc.psum_tensor([R, SH], f32))       # 1 bank
        o_ps = [E(nc.psum_tensor(f"o_ps{i}", [128, 512], f32))
                for i in range(5)]                     # 5 banks
        dma_sem = E(nc.semaphore("dma_sem"))
        w_sem = E(nc.semaphore("w_sem"))
        pe_sem = E(nc.semaphore("pe_sem"))
        dve_sem = E(nc.semaphore("dve_sem"))
        act_sem = E(nc.semaphore("act_sem"))
        x_sem = [E(nc.semaphore(f"x_sem{j}")) for j in range(4)]
        xt_sem = [E(nc.semaphore(f"xt_sem{j}")) for j in range(2)]
        block = E(nc.Block(no_gpsimd_drain=True))

        sw = lambda k: w_t.ap()[:, k * R:(k + 1) * R]
        hw = lambda k: w_t.ap()[:, KT * R + k * R:KT * R + (k + 1) * R]

        # PE program order (pe_sem counts, real matmuls only):
        #   [x0] hid k0,k1      -> 1..4   (n0,n1 per k)
        #   [xt0] seq k0..3     -> 5..8
        #   [x1] hid k2,k3      -> 9..12
        #   [xt1] seq k4..7     -> 13..16  (seq stop at k7)
        #   [x3] hid k6,k7      -> 17..20
        #   [x2] hid k4,k5      -> 21..24  (hid stop; n0 done @23, n1 @24)
        #   out j=0..7          -> 25..32
        HID_N0_DONE = 23
        HID_N1_DONE = 24
        SEQ_DONE = 16

        @block.sync
        def _(sync):
            sync.dma_start(out=w_t.ap(), in_=w_d[:]).then_inc(w_sem, 16)
            for c in range(3):  # x chunks 0..2 (k-tiles 2c, 2c+1)
                sync.dma_start(
                    out=x_t.ap()[:, 2 * c:2 * c + 2, :],
                    in_=x_d[:, c * 2 * H:(c + 1) * 2 * H],
                ).then_inc(x_sem[c], 16)
            # out chunks: m-tile ready when even copy (dve) + odd copy (act)
            # for that m have both landed
            for m in range(MT):
                sync.wait_ge(dve_sem, 3 + m)
                sync.wait_ge(act_sem, 2 + m)
                sync.dma_start(
                    out=out_d[:, m * H:(m + 1) * H],
                    in_=o_sb.ap()[:, m, :],
                ).then_inc(dma_sem, 16)
            sync.wait_ge(dma_sem, 16 * MT)

        @block.scalar
        def _(scalar):
            # Activation HWDGE queue: xt chunks + the last x chunk
            for c in range(2):  # xt chunks (k-tiles 4c..4c+3)
                scalar.dma_start(
                    out=xt_t.ap()[:, 4 * c:4 * c + 4, :],
                    in_=xt_d[:, c * 4 * SH:(c + 1) * 4 * SH],
                ).then_inc(xt_sem[c], 16)
            scalar.dma_start(
                out=x_t.ap()[:, 6:8, :],
                in_=x_d[:, 6 * H:8 * H],
            ).then_inc(x_sem[3], 16)
            # dummy copy to pull the lazy ACT table load off the critical path
            scalar.wait_ge(w_sem, 16)
            nc.scalar.copy(o_sb.ap()[:, 0, 0:R], w_t.ap()[:, 0:R])
            scalar.wait_ge(pe_sem, HID_N1_DONE)
            nc.scalar.copy(
                hid_sb.ap()[:, 512:1024],
                hid_ps.ap()[:, 512:1024]).then_inc(act_sem, 1)
            for j in range(1, 2 * MT, 2):   # odd out copies
                m, n = divmod(j, 2)
                scalar.wait_ge(pe_sem, 24 + j + 1)
                nc.scalar.copy(
                    o_sb.ap()[:, m, n * 512:(n + 1) * 512],
                    o_ps[j % 5].ap(),
                ).then_inc(act_sem, 1)

        @block.tensor
        def _(tensor):
            tensor.wait_ge(w_sem, 16)
            for _ in range(4):  # PE p-state warmup
                nc.tensor.matmul(o_ps[0].ap()[0:R, :], w_t.ap()[:, 0:R],
                                 w_t.ap()[:, 0:512], start=True, stop=True)

            def seq_k(k):
                nc.tensor.matmul(
                    seq_ps.ap(), sw(k), xt_t.ap()[:, k, :],
                    start=(k == 0), stop=(k == KT - 1),
                ).then_inc(pe_sem, 1)

            tensor.wait_ge(x_sem[0], 16)
            for k in (0, 1):
                for n in range(2):
                    nc.tensor.matmul(
                        hid_ps.ap()[:, n * 512:(n + 1) * 512],
                        hw(k), x_t.ap()[:, k, n * 512:(n + 1) * 512],
                        start=(k == 0), stop=False,
                    ).then_inc(pe_sem, 1)
            tensor.wait_ge(xt_sem[0], 16)
            for k in range(4):
                seq_k(k)
            tensor.wait_ge(x_sem[1], 16)
            for k in (2, 3):
                for n in range(2):
                    nc.tensor.matmul(
                        hid_ps.ap()[:, n * 512:(n + 1) * 512],
                        hw(k), x_t.ap()[:, k, n * 512:(n + 1) * 512],
                        start=False, stop=False,
                    ).then_inc(pe_sem, 1)
            tensor.wait_ge(xt_sem[1], 16)
            for k in range(4, 8):
                seq_k(k)
            tensor.wait_ge(x_sem[3], 16)
            for k in (6, 7):
                for n in range(2):
                    nc.tensor.matmul(
                        hid_ps.ap()[:, n * 512:(n + 1) * 512],
                        hw(k), x_t.ap()[:, k, n * 512:(n + 1) * 512],
                        start=False, stop=False,
                    ).then_inc(pe_sem, 1)
            tensor.wait_ge(x_sem[2], 16)
            for k in (4, 5):
                for n in range(2):
                    nc.tensor.matmul(
                        hid_ps.ap()[:, n * 512:(n + 1) * 512],
                        hw(k), x_t.ap()[:, k, n * 512:(n + 1) * 512],
                        start=False, stop=(k == 5),
                    ).then_inc(pe_sem, 1)

            # final stage: out[j] = seq_sb[:, m-chunk].T @ hid_sb[:, n-chunk]
            tensor.wait_ge(dve_sem, 2)   # seq copy + hid n0 copy
            for j in range(2 * MT):
                m, n = divmod(j, 2)
                if j == 1:
                    tensor.wait_ge(act_sem, 1)   # hid n1 copy (odd n only)
                if j >= 5:
                    # WAR on recycled PSUM bank (5-deep rotation)
                    prev = j - 5
                    if prev % 2 == 0:
                        tensor.wait_ge(dve_sem, 3 + prev // 2)
                    else:
                        tensor.wait_ge(act_sem, 2 + (prev - 1) // 2)
                nc.tensor.matmul(
                    o_ps[j % 5].ap(),
                    seq_sb.ap()[:, m * 128:(m + 1) * 128],
                    hid_sb.ap()[:, n * 512:(n + 1) * 512],
                    start=True, stop=True,
                ).then_inc(pe_sem, 1)

        @block.vector
        def _(vector):
            vector.wait_ge(pe_sem, SEQ_DONE)
            nc.vector.tensor_copy(
                seq_sb.ap(), seq_ps.ap()).then_inc(dve_sem, 1)
            vector.wait_ge(pe_sem, HID_N0_DONE)
            nc.vector.tensor_copy(
                hid_sb.ap()[:, 0:512],
                hid_ps.ap()[:, 0:512]).then_inc(dve_sem, 1)
            for j in range(0, 2 * MT, 2):   # even out copies
                m, n = divmod(j, 2)
                vector.wait_ge(pe_sem, 24 + j + 1)
                nc.vector.tensor_copy(
                    o_sb.ap()[:, m, n * 512:(n + 1) * 512],
                    o_ps[j % 5].ap(),
                ).then_inc(dve_sem, 1)

    return nc


def _get_program():
    if "nc" not in _compiled:
        _compiled["nc"] = build_raw_program()
    return _compiled["nc"]


def _make_in_maps(hidden_states, seq_W, hid_W, cp_weight):
    swT = _wtile(np.ascontiguousarray(seq_W.T, dtype=np.float16))  # [128, 256]
    hwT_rows = np.ascontiguousarray(
        (hid_W * cp_weight[0][:, None]).T, dtype=np.float16)       # [S, R]
    # per-half row rotation: own seq half first (hid contraction over S is
    # order-invariant as long as x rows and hw rows permute together)
    w_rot = [
        np.ascontiguousarray(np.concatenate([swT, _wtile(np.concatenate(
            [hwT_rows[half * SH:], hwT_rows[:half * SH]], axis=0))], axis=1))
        for half in range(2)
    ]
    in_maps = []
    for c in range(N_CORES):
        b, half = divmod(c, 2)
        xb = hidden_states[b].astype(np.float16)
        if half:
            xb = np.concatenate([xb[SH:], xb[:SH]], axis=0)
        in_maps.append({
            "x": _tile128(xb),
            "xt": _tile128(np.ascontiguousarray(xb[:SH, :].T)),
            "w": w_rot[half],
        })
    return in_maps


def kernel(hidden_states, all_indices, seq_W, hid_W, cp_weight):
    hidden_states = np.asarray(hidden_states, dtype=np.float32)
    seq_W = np.asarray(seq_W, dtype=np.float32)
    hid_W = np.asarray(hid_W, dtype=np.float32)
    cp_weight = np.asarray(cp_weight, dtype=np.float32)
    idx = np.asarray(all_indices)

    # The reference's all_indices is always the full cartesian grid; verify
    # cheaply and fall back to a host path if ever not.
    n = np.arange(S * H, dtype=idx.dtype)
    if idx.shape != (S * H, 2) or not (
        np.array_equal(idx[:, 0], n // H) and np.array_equal(idx[:, 1], n % H)
    ):
        return _np_fallback(hidden_states, idx, seq_W, hid_W, cp_weight)

    from concourse.bass_utils import run_bass_kernel_spmd

    nc = _get_program()
    in_maps = _make_in_maps(hidden_states, seq_W, hid_W, cp_weight)
    res = run_bass_kernel_spmd(nc, in_maps, list(range(N_CORES)))

    out = np.empty((B, S, H), dtype=np.float32)
    for c in range(N_CORES):
        b, half = divmod(c, 2)
        ot = res.results[c]["out"]  # [128, MT*H] f16, tiled
        out[b, half * SH:(half + 1) * SH, :] = (
            ot.reshape(128, MT, H).transpose(1, 0, 2).reshape(SH, H)
            .astype(np.float32))
    return out
